# revision 13
# baseline (speedup 1.0000x reference)
"""Trainium2 Bass kernel for DWT linear attention (nn_DWTLinearAttention).

Shards the 4 batch samples x 2 independent streams (x / y) across the 8
NeuronCores: core b handles x[b], core 4+b handles y[b].  Each core runs
the full per-sample pipeline:

  FLAT (C=512, N=16384) view of the (N, C) input buffer
  ll' = a+b+c+d  (2x2 haar low-pass, unscaled)                (DVE)
  Qpre = wq/2 @ ll' + bq ; column-l2-normalize -> Qn          (PE + DVE/ACT)
  KT/VT = ll'^T @ [wk/2 | wv/2]^T + bias (transposed layout)  (PE)
  KnT row-normalized; matrix' = [Kn;1]^T VT; ksum; tailor     (PE + DVE/ACT)
  P' = [Qn;1]^T-chunk @ matrix' ; pscal = P' * tailor         (PE + DVE/ACT)
  out[n', c] = x^T + S_att @ pscal + S_ll @ ll'^T             (PE transposes +
               dup-pattern matmuls accumulated in PSUM)
  where S_att bakes 0.5*gamma and S_ll bakes -0.25 (out = x + 0.5*(att-ll)).

Matmuls run in float32r mode (full-rate fp32 PE streaming).
"""

import os
import sys

for _p in ("/opt/trn_rl_repo", "/root/.axon_site/_ro/trn_rl_repo"):
    if _p not in sys.path and os.path.isdir(_p):
        sys.path.append(_p)

import numpy as np

import concourse.bass as bass
import concourse.tile as tile
from concourse import bacc, mybir
from concourse import bass_utils

F32 = mybir.dt.float32
F32R = mybir.dt.float32r
AF = mybir.ActivationFunctionType
ALU = mybir.AluOpType
ts = bass.ts

C = 512
N = 16384
W = 128          # spatial rows of the (C, W, H) view
H = 128
NL = 4096        # low-band spatial size (64*64)
M = 64           # attention inner dim
EPS = 1e-6

USE_F32R = True


def _r(ap):
    return ap.bitcast(F32R) if USE_F32R else ap


def build_program():
    nc = bacc.Bacc(
        "TRN2",
        target_bir_lowering=False,
        debug=False,
        enable_asserts=True,
        num_devices=8,
    )

    d_x = nc.dram_tensor("xb", [C, N], F32, kind="ExternalInput").ap()
    d_wqT = nc.dram_tensor("wqT", [C, 128], F32, kind="ExternalInput").ap()
    d_wkT = nc.dram_tensor("wkT", [C, M], F32, kind="ExternalInput").ap()
    d_wvT = nc.dram_tensor("wvT", [C, C], F32, kind="ExternalInput").ap()
    d_bq = nc.dram_tensor("bq", [M, 1], F32, kind="ExternalInput").ap()
    d_bkb = nc.dram_tensor("bkb", [128, M], F32, kind="ExternalInput").ap()
    d_bvb = nc.dram_tensor("bvb", [128, C], F32, kind="ExternalInput").ap()
    d_eye = nc.dram_tensor("eye", [128, 128], F32, kind="ExternalInput").ap()
    d_satt = nc.dram_tensor("satt", [M, 128], F32, kind="ExternalInput").ap()
    d_sll = nc.dram_tensor("sll", [M, 128], F32, kind="ExternalInput").ap()
    d_onesP = nc.dram_tensor("onesP", [128, 128], F32,
                             kind="ExternalInput").ap()
    d_out = nc.dram_tensor("out", [N, C], F32, kind="ExternalOutput").ap()

    with tile.TileContext(nc) as tc:
        _emit(nc, tc, d_x, d_wqT, d_wkT, d_wvT, d_bq, d_bkb, d_bvb,
              d_eye, d_satt, d_sll, d_onesP, d_out)

    nc.compile()
    return nc


def _emit(nc, tc, d_x, d_wqT, d_wkT, d_wvT, d_bq, d_bkb, d_bvb,
          d_eye, d_satt, d_sll, d_onesP, d_out):
    from contextlib import ExitStack
    ctx = ExitStack()
    with ctx:
        ctx.enter_context(
            nc.allow_low_precision(reason="f32r rounding for PE matmuls"))
        # ---------------- pools ----------------
        # PSUM: exactly 8 banks total.
        pp1 = ctx.enter_context(tc.tile_pool(name="pp1", bufs=3, space="PSUM"))
        pp2 = ctx.enter_context(tc.tile_pool(name="pp2", bufs=2, space="PSUM"))
        pp3 = ctx.enter_context(tc.tile_pool(name="pp3", bufs=1, space="PSUM"))
        ppM = ctx.enter_context(tc.tile_pool(name="ppM", bufs=1, space="PSUM"))
        ppKS = ctx.enter_context(tc.tile_pool(name="ppKS", bufs=1, space="PSUM"))

        cpool = ctx.enter_context(tc.tile_pool(name="consts", bufs=1))
        llpool = ctx.enter_context(tc.tile_pool(name="ll", bufs=4))
        qnpool = ctx.enter_context(tc.tile_pool(name="qn", bufs=1))
        xpool = ctx.enter_context(tc.tile_pool(name="xin", bufs=2))
        t1pool = ctx.enter_context(tc.tile_pool(name="t1", bufs=2))
        sqpool = ctx.enter_context(tc.tile_pool(name="sq", bufs=2))
        nrmpool = ctx.enter_context(tc.tile_pool(name="nrm", bufs=2))
        bcpool = ctx.enter_context(tc.tile_pool(name="bc", bufs=2))
        kpool = ctx.enter_context(tc.tile_pool(name="kpre", bufs=2))
        kntpool = ctx.enter_context(tc.tile_pool(name="knt", bufs=2))
        vtpool = ctx.enter_context(tc.tile_pool(name="vt", bufs=3))
        mspool = ctx.enter_context(tc.tile_pool(name="ms", bufs=1))
        stpool = ctx.enter_context(tc.tile_pool(name="st", bufs=2))
        pspool = ctx.enter_context(tc.tile_pool(name="pscal", bufs=3))
        llTpool = ctx.enter_context(tc.tile_pool(name="llT", bufs=3))
        xwpool = ctx.enter_context(tc.tile_pool(name="xw", bufs=4))
        opool = ctx.enter_context(tc.tile_pool(name="outs", bufs=3))

        # ---------------- constants into SBUF ----------------
        # non-matmul constants live in cpool as plain f32
        bq_sb = cpool.tile([M, 1], F32, tag="bq")
        nc.sync.dma_start(bq_sb[:], d_bq)
        bkb_sb = cpool.tile([128, M], F32, tag="bkb")
        nc.sync.dma_start(bkb_sb[:], d_bkb)
        bvb_sb = cpool.tile([128, C], F32, tag="bvb")
        nc.sync.dma_start(bvb_sb[:], d_bvb)
        eye_sb = cpool.tile([128, 128], F32, tag="eye")
        nc.sync.dma_start(eye_sb[:], d_eye)
        onesP_sb = cpool.tile([128, 128], F32, tag="onesP")
        nc.sync.dma_start(onesP_sb[:], d_onesP)

        # matmul-consumed constants: DMA into rotating scratch, then
        # round into persistent f32r tiles (fp32r requires producer
        # rounding, which DMA cannot do).
        def _load_r(dst_tag, shape, src_ap, scratch_pool, scratch_tag,
                    scratch_shape, blocked=None):
            t = cpool.tile(shape, F32, tag=dst_tag, name=dst_tag)
            stg = scratch_pool.tile(scratch_shape, F32,
                                    tag=scratch_tag, name=dst_tag + "_stg")
            view = stg[0:shape[0], 0:shape[1]]
            if blocked:
                nc.sync.dma_start(
                    view.rearrange("p (cb m) -> p cb m", cb=4), src_ap)
            else:
                nc.sync.dma_start(view, src_ap)
            nc.vector.tensor_copy(t[:].bitcast(F32R), view)
            return t

        wqT_r = _load_r("wqT_r", [128, 4 * 128],
                        d_wqT.rearrange("(cb p) m -> p cb m", p=128),
                        xpool, "xt", [128, 2048], blocked=True)
        wkT_r = _load_r("wkT_r", [128, 4 * M],
                        d_wkT.rearrange("(cb p) m -> p cb m", p=128),
                        xpool, "xt", [128, 2048], blocked=True)
        wvT_r = _load_r("wvT_r", [128, 4 * C],
                        d_wvT.rearrange("(cb p) m -> p cb m", p=128),
                        xpool, "xt", [128, 2048], blocked=True)
        satt_r = _load_r("satt_r", [M, 128], d_satt, t1pool, "t1",
                         [128, 1024])
        sll_r = _load_r("sll_r", [M, 128], d_sll, t1pool, "t1", [128, 1024])
        onesP_r = cpool.tile([128, 128], F32, tag="onesP_r")
        nc.vector.tensor_copy(onesP_r[:].bitcast(F32R), onesP_sb[:])

        # ---------------- phase 1: ll' = a+b+c+d ----------------
        ll_t = [llpool.tile([128, NL], F32, tag="ll", name=f"ll{i}")
                for i in range(4)]
        for cb in range(4):
            for ws in range(8):  # strips of 16 w-rows
                xt = xpool.tile([128, 2048], F32, tag="xt")
                nc.sync.dma_start(
                    xt[:], d_x[ts(cb, 128), ws * 2048:(ws + 1) * 2048])
                # pair along h: (16w, 128h) -> (16w, 64)
                xv = xt[:].rearrange("p (a t) -> p a t", t=2)
                t1 = t1pool.tile([128, 1024], F32, tag="t1")
                nc.vector.tensor_add(t1[:], xv[:, :, 0:1], xv[:, :, 1:2])
                # pair along w: (16w, 64) -> (8i, 64)
                tv = t1[:].rearrange("p (i t j) -> p i t j", t=2, j=64)
                nc.vector.tensor_add(
                    ll_t[cb][:, ws * 512:(ws + 1) * 512].bitcast(F32R),
                    tv[:, :, 0:1, :], tv[:, :, 1:2, :])

        # ---------------- phase 2: Q path (64, NL), normalized ----------
        qn_t = qnpool.tile([M + 1, NL], F32, tag="qn")
        qrow = cpool.tile([1, 512], F32, tag="qrow")
        nc.vector.memset(qrow[:], 1.0)
        for qc in range(8):
            nc.vector.tensor_copy(qn_t[M:M + 1, ts(qc, 512)].bitcast(F32R),
                                  qrow[:])
        for qc in range(8):
            psQ = pp1.tile([128, 512], F32, tag="a")
            for cb in range(4):
                nc.tensor.matmul(
                    psQ[:],
                    _r(wqT_r[:, ts(cb, 128)]),
                    _r(ll_t[cb][:, ts(qc, 512)]),
                    start=(cb == 0), stop=(cb == 3))
            sq = sqpool.tile([M, 512], F32, tag="sq")
            nc.scalar.activation(sq[:].bitcast(F32R), psQ[0:M, :], AF.Square,
                                 bias=bq_sb[:, 0:1], scale=1.0)
            psSS = pp3.tile([128, 512], F32, tag="c")
            nc.tensor.matmul(psSS[:], _r(onesP_r[0:M, :]), _r(sq[:]),
                             start=True, stop=True)
            nrm = nrmpool.tile([1, 512], F32, tag="nrm")
            nc.scalar.sqrt(nrm[:], psSS[0:1, :])
            inv = nrmpool.tile([1, 512], F32, tag="inv")
            nc.vector.reciprocal(inv[:].bitcast(F32R), nrm[:])
            psB = pp2.tile([128, 512], F32, tag="b")
            nc.tensor.matmul(psB[:], _r(onesP_r[0:1, :]), _r(inv[:]),
                             start=True, stop=True)
            bcs = bcpool.tile([M, 512], F32, tag="bcs")
            nc.scalar.copy(bcs[:], psB[0:M, :])
            # qn = (psQ + bq) * bcast(1/norm)
            nc.vector.scalar_tensor_tensor(
                qn_t[0:M, ts(qc, 512)].bitcast(F32R), psQ[0:M, :],
                bq_sb[:, 0:1], bcs[:], op0=ALU.add, op1=ALU.mult)

        # ---------------- phase 3: K/V transposed, matrix', ksum --------
        psM = ppM.tile([M + 1, 512], F32, tag="m")
        psKS = ppKS.tile([M, 1], F32, tag="ks")
        for kc in range(32):
            psK = pp2.tile([128, M], F32, tag="b")
            psV = pp1.tile([128, 512], F32, tag="a")
            for cb in range(4):
                nc.tensor.matmul(
                    psK[:],
                    _r(ll_t[cb][:, ts(kc, 128)]),
                    _r(wkT_r[:, ts(cb, M)]),
                    start=(cb == 0), stop=(cb == 3))
            for cb in range(4):
                nc.tensor.matmul(
                    psV[:],
                    _r(ll_t[cb][:, ts(kc, 128)]),
                    _r(wvT_r[:, ts(cb, C)]),
                    start=(cb == 0), stop=(cb == 3))
            kpre = kpool.tile([128, M], F32, tag="kpre")
            nc.vector.tensor_add(kpre[:], psK[:], bkb_sb[:])
            scr = kpool.tile([128, M], F32, tag="scr")
            ssq = stpool.tile([128, 1], F32, tag="ssq")
            nc.scalar.activation(scr[:], kpre[:], AF.Square,
                                 accum_out=ssq[:])
            nrm2 = stpool.tile([128, 1], F32, tag="nrm2")
            nc.scalar.sqrt(nrm2[:], ssq[:])
            ik = stpool.tile([128, 1], F32, tag="ik")
            nc.vector.reciprocal(ik[:], nrm2[:])
            knt = kntpool.tile([128, M + 1], F32, tag="knt")
            nc.vector.tensor_copy(knt[:, M:M + 1].bitcast(F32R), onesP_sb[:, 0:1])
            nc.vector.tensor_scalar_mul(knt[:, 0:M].bitcast(F32R), kpre[:], ik[:, 0:1])
            vt = vtpool.tile([128, 512], F32, tag="vt")
            nc.vector.tensor_add(vt[:].bitcast(F32R), psV[:], bvb_sb[:])
            nc.tensor.matmul(psM[:], _r(knt[:]), _r(vt[:]),
                             start=(kc == 0), stop=(kc == 31))
            nc.tensor.matmul(psKS[:], knt[:, 0:M], onesP_sb[:, 0:1],
                             start=(kc == 0), stop=(kc == 31))

        matrix_sb = mspool.tile([M + 1, 512], F32, tag="ms")
        nc.vector.tensor_copy(matrix_sb[:].bitcast(F32R), psM[:])
        ksum_sb = mspool.tile([M + 1, 1], F32, tag="ksum")
        nc.vector.tensor_scalar_mul(ksum_sb[M:M + 1, :].bitcast(F32R),
                                    onesP_sb[0:1, 0:1], float(NL))
        nc.vector.tensor_scalar_add(ksum_sb[0:M, :].bitcast(F32R), psKS[:], EPS)

        # ---------------- phases 4+5 interleaved ------------------------
        for jc in range(32):
            psP = pp1.tile([128, 512], F32, tag="a")
            nc.tensor.matmul(psP[:], _r(qn_t[:, ts(jc, 128)]),
                             _r(matrix_sb[:]), start=True, stop=True)
            psT = pp3.tile([128, 1], F32, tag="c")
            nc.tensor.matmul(psT[:], qn_t[:, ts(jc, 128)],
                             ksum_sb[:], start=True, stop=True)
            sT = stpool.tile([128, 1], F32, tag="sT")
            nc.vector.reciprocal(sT[:], psT[:])
            # pscal halves (base-0 tiles for the assembly matmuls)
            ps_a = pspool.tile([M, 512], F32, tag="ps_a")
            nc.scalar.mul(ps_a[:].bitcast(F32R), psP[0:M, :], sT[0:M, 0:1])
            ps_b = pspool.tile([M, 512], F32, tag="ps_b")
            nc.vector.tensor_scalar_mul(ps_b[:].bitcast(F32R),
                                        psP[M:128, :], sT[M:128, 0:1])
            # ll'^T chunk via PE transposes
            psL = pp2.tile([128, 512], F32, tag="b")
            for cb in range(4):
                nc.tensor.matmul(psL[:, ts(cb, 128)],
                                 ll_t[cb][:, ts(jc, 128)], eye_sb[:],
                                 is_transpose=True,
                                 start=True, stop=True,
                                 skip_group_check=True)
            llT_a = llTpool.tile([M, 512], F32, tag="llT_a")
            nc.scalar.copy(llT_a[:].bitcast(F32R), psL[0:M, :])
            llT_b = llTpool.tile([M, 512], F32, tag="llT_b")
            nc.vector.tensor_copy(llT_b[:].bitcast(F32R), psL[M:128, :])

            # ---- 4 output chunks per jc ----
            for wi in range(4):
                w = 4 * jc + wi
                rhs_att = ps_a if wi < 2 else ps_b
                rhs_ll = llT_a if wi < 2 else llT_b
                xw = xwpool.tile([128, 512], F32, tag="xw")
                nc.sync.dma_start(
                    xw[:].rearrange("p (cb h) -> p cb h", cb=4),
                    d_x.rearrange("(cb p) n -> p cb n", p=128)[
                        :, :, w * 128:(w + 1) * 128])
                psO = pp1.tile([128, 512], F32, tag="a")
                nc.tensor.matmul(psO[:], _r(satt_r[:]), _r(rhs_att[:]),
                                 start=True, stop=False,
                                 skip_group_check=True)
                nc.tensor.matmul(psO[:], _r(sll_r[:]), _r(rhs_ll[:]),
                                 start=False, stop=False,
                                 skip_group_check=True)
                for cb in range(4):
                    nc.tensor.matmul(psO[:, ts(cb, 128)],
                                     xw[:, ts(cb, 128)], eye_sb[:],
                                     is_transpose=True,
                                     start=False, stop=(cb == 3),
                                     skip_group_check=True)
                out_s = opool.tile([128, 512], F32, tag="outs")
                if w % 2 == 0:
                    nc.vector.tensor_copy(out_s[:], psO[:])
                else:
                    nc.scalar.copy(out_s[:], psO[:])
                nc.sync.dma_start(d_out[w * 128:(w + 1) * 128, :], out_s[:])


# ------------------------------------------------------------------
# host-side wrapper
# ------------------------------------------------------------------
_NC_CACHE = None


def _get_program():
    global _NC_CACHE
    if _NC_CACHE is None:
        _NC_CACHE = build_program()
    return _NC_CACHE


def _make_in_map(xb, wq, bq, wk, bk, wv, bv, gamma):
    dup = np.zeros((M, 128), dtype=np.float32)
    for j in range(M):
        dup[j, 2 * j] = 1.0
        dup[j, 2 * j + 1] = 1.0
    g = float(np.asarray(gamma).reshape(-1)[0])
    wqT = np.zeros((C, 128), dtype=np.float32)
    wqT[:, 0:M] = (0.5 * wq).T
    return {
        "xb": np.ascontiguousarray(xb.reshape(C, N)),
        "wqT": wqT,
        "wkT": np.ascontiguousarray((0.5 * wk).T),
        "wvT": np.ascontiguousarray((0.5 * wv).T),
        "bq": np.ascontiguousarray(bq.reshape(M, 1)),
        "bkb": np.ascontiguousarray(np.broadcast_to(bk[None, :], (128, M))),
        "bvb": np.ascontiguousarray(np.broadcast_to(bv[None, :], (128, C))),
        "eye": np.eye(128, dtype=np.float32),
        "satt": np.ascontiguousarray(0.5 * g * dup),
        "sll": np.ascontiguousarray(-0.25 * dup),
        "onesP": np.ones((128, 128), dtype=np.float32),
    }


def kernel(x, y, gamma, gamma_y, wq, bq, wk, bk, wv, bv,
           wqy, bqy, wky, bky, wvy, bvy):
    x = np.asarray(x, dtype=np.float32)
    y = np.asarray(y, dtype=np.float32)
    B = x.shape[0]
    assert x.shape == (B, N, C), x.shape

    nc = _get_program()
    in_maps = []
    for b in range(B):
        in_maps.append(_make_in_map(x[b], wq, bq, wk, bk, wv, bv, gamma))
    for b in range(B):
        in_maps.append(_make_in_map(y[b], wqy, bqy, wky, bky, wvy, bvy,
                                    gamma_y))
    res = bass_utils.run_bass_kernel_spmd(
        nc, in_maps, core_ids=list(range(8)))
    out_x = np.stack([res.results[b]["out"] for b in range(B)])
    out_y = np.stack([res.results[B + b]["out"] for b in range(B)])
    return (out_x, out_y)


# revision 14
# speedup vs baseline: 1.1307x; 1.1307x over previous
"""Trainium2 Bass kernel for DWT linear attention (nn_DWTLinearAttention).

Shards the 4 batch samples x 2 independent streams (x / y) across the 8
NeuronCores: core b handles x[b], core 4+b handles y[b].  Each core runs
the full per-sample pipeline:

  FLAT (C=512, N=16384) view of the (N, C) input buffer
  ll' = a+b+c+d  (2x2 haar low-pass, unscaled)                (DVE)
  Qpre = wq/2 @ ll' + bq ; column-l2-normalize -> Qn          (PE + DVE/ACT)
  KT/VT = ll'^T @ [wk/2 | wv/2]^T + bias (transposed layout)  (PE)
  KnT row-normalized; matrix' = [Kn;1]^T VT; ksum; tailor     (PE + DVE/ACT)
  P' = [Qn;1]^T-chunk @ matrix' ; pscal = P' * tailor         (PE + DVE/ACT)
  out[n', c] = x^T + Scomb @ [pscal ; ll'^T]                  (PE transposes +
               one dup-pattern matmul accumulated in PSUM)
  where Scomb bakes 0.5*gamma (att rows) and -0.25 (ll rows), from
  out = x + 0.5*(att - ll).

Heavy matmuls run in float32r mode (full-rate fp32 PE streaming); fp32r
requires producers to round their outputs (bitcast(F32R) on out APs) and
is restricted to full 128-column tiling with even innermost counts, so
tiny N=1 / M<128 matmuls use plain fp32 or padded operands.

Phases 2+3 are interleaved with the phase-1 input stream (subtile deps
let QKV matmuls start as soon as the needed ll slices are written), and
phase 5's x re-read prefetches during phase 4.
"""

import os
import sys

for _p in ("/opt/trn_rl_repo", "/root/.axon_site/_ro/trn_rl_repo"):
    if _p not in sys.path and os.path.isdir(_p):
        sys.path.append(_p)

import numpy as np

import concourse.bass as bass
import concourse.tile as tile
from concourse import bacc, mybir
from concourse import bass_utils

F32 = mybir.dt.float32
F32R = mybir.dt.float32r
AF = mybir.ActivationFunctionType
ALU = mybir.AluOpType
ts = bass.ts

C = 512
N = 16384
NL = 4096        # low-band spatial size (64*64)
M = 64           # attention inner dim
EPS = 1e-6

USE_F32R = True


def _r(ap):
    return ap.bitcast(F32R) if USE_F32R else ap


def build_program():
    nc = bacc.Bacc(
        "TRN2",
        target_bir_lowering=False,
        debug=False,
        enable_asserts=True,
        num_devices=8,
    )

    d = {}
    d["xb"] = nc.dram_tensor("xb", [C, N], F32, kind="ExternalInput").ap()
    d["wqT"] = nc.dram_tensor("wqT", [C, 128], F32, kind="ExternalInput").ap()
    d["wkT"] = nc.dram_tensor("wkT", [C, M], F32, kind="ExternalInput").ap()
    d["wvT"] = nc.dram_tensor("wvT", [C, C], F32, kind="ExternalInput").ap()
    d["bq"] = nc.dram_tensor("bq", [M, 1], F32, kind="ExternalInput").ap()
    d["bkb"] = nc.dram_tensor("bkb", [128, M], F32, kind="ExternalInput").ap()
    d["bvb"] = nc.dram_tensor("bvb", [128, C], F32, kind="ExternalInput").ap()
    d["eye"] = nc.dram_tensor("eye", [128, 128], F32, kind="ExternalInput").ap()
    d["scomb"] = nc.dram_tensor("scomb", [128, 128], F32,
                                kind="ExternalInput").ap()
    d["onesP"] = nc.dram_tensor("onesP", [128, 128], F32,
                                kind="ExternalInput").ap()
    d["out"] = nc.dram_tensor("out", [N, C], F32, kind="ExternalOutput").ap()

    with tile.TileContext(nc) as tc:
        _emit(nc, tc, d)

    nc.compile()
    return nc


def _emit(nc, tc, d):
    from contextlib import ExitStack
    ctx = ExitStack()
    with ctx:
        ctx.enter_context(
            nc.allow_low_precision(reason="f32r rounding for PE matmuls"))
        # ---------------- pools (PSUM: exactly 8 banks) ----------------
        pp1 = ctx.enter_context(tc.tile_pool(name="pp1", bufs=3, space="PSUM"))
        pp2 = ctx.enter_context(tc.tile_pool(name="pp2", bufs=2, space="PSUM"))
        pp3 = ctx.enter_context(tc.tile_pool(name="pp3", bufs=1, space="PSUM"))
        ppM = ctx.enter_context(tc.tile_pool(name="ppM", bufs=1, space="PSUM"))
        ppKS = ctx.enter_context(tc.tile_pool(name="ppKS", bufs=1,
                                              space="PSUM"))

        cpool = ctx.enter_context(tc.tile_pool(name="consts", bufs=1))
        llpool = ctx.enter_context(tc.tile_pool(name="ll", bufs=4))
        qnpool = ctx.enter_context(tc.tile_pool(name="qn", bufs=1))
        xpool = ctx.enter_context(tc.tile_pool(name="xin", bufs=2))
        t1pool = ctx.enter_context(tc.tile_pool(name="t1", bufs=2))
        sqpool = ctx.enter_context(tc.tile_pool(name="sq", bufs=2))
        nrmpool = ctx.enter_context(tc.tile_pool(name="nrm", bufs=2))
        bcpool = ctx.enter_context(tc.tile_pool(name="bc", bufs=2))
        kpool = ctx.enter_context(tc.tile_pool(name="kpre", bufs=2))
        kntpool = ctx.enter_context(tc.tile_pool(name="knt", bufs=2))
        vtpool = ctx.enter_context(tc.tile_pool(name="vt", bufs=3))
        mspool = ctx.enter_context(tc.tile_pool(name="ms", bufs=1))
        stpool = ctx.enter_context(tc.tile_pool(name="st", bufs=2))
        cbpool = ctx.enter_context(tc.tile_pool(name="comb", bufs=4))
        xwpool = ctx.enter_context(tc.tile_pool(name="xw", bufs=8))
        opool = ctx.enter_context(tc.tile_pool(name="outs", bufs=3))

        # ---------------- constants ----------------
        bq_sb = cpool.tile([M, 1], F32, tag="bq")
        nc.sync.dma_start(bq_sb[:], d["bq"])
        bkb_sb = cpool.tile([128, M], F32, tag="bkb")
        nc.sync.dma_start(bkb_sb[:], d["bkb"])
        bvb_sb = cpool.tile([128, C], F32, tag="bvb")
        nc.sync.dma_start(bvb_sb[:], d["bvb"])
        eye_sb = cpool.tile([128, 128], F32, tag="eye")
        nc.sync.dma_start(eye_sb[:], d["eye"])
        onesP_sb = cpool.tile([128, 128], F32, tag="onesP")
        nc.sync.dma_start(onesP_sb[:], d["onesP"])

        # matmul-consumed constants: DMA into rotating scratch, then round
        # into persistent f32r tiles (fp32r needs producer-side rounding,
        # which DMA cannot do).
        def _load_r(dst_tag, shape, src_ap, scratch_pool, scratch_tag,
                    scratch_shape, blocked=False):
            t = cpool.tile(shape, F32, tag=dst_tag, name=dst_tag)
            stg = scratch_pool.tile(scratch_shape, F32,
                                    tag=scratch_tag, name=dst_tag + "_stg")
            view = stg[0:shape[0], 0:shape[1]]
            if blocked:
                nc.sync.dma_start(
                    view.rearrange("p (cb m) -> p cb m", cb=4), src_ap)
            else:
                nc.sync.dma_start(view, src_ap)
            nc.vector.tensor_copy(t[:].bitcast(F32R), view)
            return t

        wqT_r = _load_r("wqT_r", [128, 4 * 128],
                        d["wqT"].rearrange("(cb p) m -> p cb m", p=128),
                        xpool, "xt", [128, 2048], blocked=True)
        wkT_r = _load_r("wkT_r", [128, 4 * M],
                        d["wkT"].rearrange("(cb p) m -> p cb m", p=128),
                        xpool, "xt", [128, 2048], blocked=True)
        wvT_r = _load_r("wvT_r", [128, 4 * C],
                        d["wvT"].rearrange("(cb p) m -> p cb m", p=128),
                        xpool, "xt", [128, 2048], blocked=True)
        scomb_r = _load_r("scomb_r", [128, 128], d["scomb"], t1pool, "t1",
                          [128, 1024])
        onesP_r = cpool.tile([128, 128], F32, tag="onesP_r")
        nc.vector.tensor_copy(onesP_r[:].bitcast(F32R), onesP_sb[:])

        ll_t = [llpool.tile([128, NL], F32, tag="ll", name=f"ll{i}")
                for i in range(4)]
        qn_t = qnpool.tile([M + 1, NL], F32, tag="qn")
        qrow = cpool.tile([1, 512], F32, tag="qrow")
        nc.vector.memset(qrow[:], 1.0)
        for qc in range(8):
            nc.vector.tensor_copy(qn_t[M:M + 1, ts(qc, 512)].bitcast(F32R),
                                  qrow[:])
        psM = ppM.tile([M + 1, 512], F32, tag="m")
        psKS = ppKS.tile([M, 1], F32, tag="ks")

        # ------- phase 1 strip: ll' = a+b+c+d for (cb, ws) -------
        def p1_strip(cb, ws):
            xt = xpool.tile([128, 2048], F32, tag="xt", name="xt")
            nc.sync.dma_start(
                xt[:], d["xb"][ts(cb, 128), ws * 2048:(ws + 1) * 2048])
            xv = xt[:].rearrange("p (a t) -> p a t", t=2)
            t1 = t1pool.tile([128, 1024], F32, tag="t1", name="t1")
            nc.vector.tensor_add(t1[:], xv[:, :, 0:1], xv[:, :, 1:2])
            tv = t1[:].rearrange("p (i t j) -> p i t j", t=2, j=64)
            nc.vector.tensor_add(
                ll_t[cb][:, ws * 512:(ws + 1) * 512].bitcast(F32R),
                tv[:, :, 0:1, :], tv[:, :, 1:2, :])

        # ------- phase 2 chunk: Qn for n-slice qc (512 wide) -------
        def p2_chunk(qc):
            psQ = pp1.tile([128, 512], F32, tag="a", name="psQ")
            for cb in range(4):
                nc.tensor.matmul(
                    psQ[:],
                    _r(wqT_r[:, ts(cb, 128)]),
                    _r(ll_t[cb][:, ts(qc, 512)]),
                    start=(cb == 0), stop=(cb == 3))
            sq = sqpool.tile([M, 512], F32, tag="sq", name="sq")
            nc.scalar.activation(sq[:].bitcast(F32R), psQ[0:M, :], AF.Square,
                                 bias=bq_sb[:, 0:1], scale=1.0)
            psSS = pp3.tile([128, 512], F32, tag="c", name="psSS")
            nc.tensor.matmul(psSS[:], _r(onesP_r[0:M, :]), _r(sq[:]),
                             start=True, stop=True)
            nrm = nrmpool.tile([1, 512], F32, tag="nrm", name="nrm")
            nc.scalar.sqrt(nrm[:], psSS[0:1, :])
            inv = nrmpool.tile([1, 512], F32, tag="inv", name="inv")
            nc.vector.reciprocal(inv[:].bitcast(F32R), nrm[:])
            psB = pp2.tile([128, 512], F32, tag="b", name="psB")
            nc.tensor.matmul(psB[:], _r(onesP_r[0:1, :]), _r(inv[:]),
                             start=True, stop=True)
            bcs = bcpool.tile([M, 512], F32, tag="bcs", name="bcs")
            nc.scalar.copy(bcs[:], psB[0:M, :])
            nc.vector.scalar_tensor_tensor(
                qn_t[0:M, ts(qc, 512)].bitcast(F32R), psQ[0:M, :],
                bq_sb[:, 0:1], bcs[:], op0=ALU.add, op1=ALU.mult)

        # ------- phase 3 chunk: KnT/VT for n-slice kc (128 wide) -------
        def p3_chunk(kc):
            psK = pp2.tile([128, M], F32, tag="b", name="psK")
            psV = pp1.tile([128, 512], F32, tag="a", name="psV")
            for cb in range(4):
                nc.tensor.matmul(
                    psK[:],
                    _r(ll_t[cb][:, ts(kc, 128)]),
                    _r(wkT_r[:, ts(cb, M)]),
                    start=(cb == 0), stop=(cb == 3))
            for cb in range(4):
                nc.tensor.matmul(
                    psV[:],
                    _r(ll_t[cb][:, ts(kc, 128)]),
                    _r(wvT_r[:, ts(cb, C)]),
                    start=(cb == 0), stop=(cb == 3))
            kpre = kpool.tile([128, M], F32, tag="kpre", name="kpre")
            nc.vector.tensor_add(kpre[:], psK[:], bkb_sb[:])
            scr = kpool.tile([128, M], F32, tag="scr", name="scr")
            ssq = stpool.tile([128, 1], F32, tag="ssq", name="ssq")
            nc.scalar.activation(scr[:], kpre[:], AF.Square,
                                 accum_out=ssq[:])
            nrm2 = stpool.tile([128, 1], F32, tag="nrm2", name="nrm2")
            nc.scalar.sqrt(nrm2[:], ssq[:])
            ik = stpool.tile([128, 1], F32, tag="ik", name="ik")
            nc.vector.reciprocal(ik[:], nrm2[:])
            knt = kntpool.tile([128, M + 1], F32, tag="knt", name="knt")
            nc.vector.tensor_copy(knt[:, M:M + 1].bitcast(F32R),
                                  onesP_sb[:, 0:1])
            nc.vector.tensor_scalar_mul(knt[:, 0:M].bitcast(F32R), kpre[:],
                                        ik[:, 0:1])
            vt = vtpool.tile([128, 512], F32, tag="vt", name="vt")
            nc.vector.tensor_add(vt[:].bitcast(F32R), psV[:], bvb_sb[:])
            nc.tensor.matmul(psM[:], _r(knt[:]), _r(vt[:]),
                             start=(kc == 0), stop=(kc == 31))
            nc.tensor.matmul(psKS[:], knt[:, 0:M], onesP_sb[:, 0:1],
                             start=(kc == 0), stop=(kc == 31))

        # ------- interleaved phases 1+2+3 -------
        for ws in range(8):
            for cb in range(4):
                p1_strip(cb, ws)
        for grp in range(8):
            p2_chunk(grp)
            for kc in range(4 * grp, 4 * grp + 4):
                p3_chunk(kc)

        # ------- phase 3.5: matrix' / ksum to SBUF -------
        matrix_sb = mspool.tile([M + 1, 512], F32, tag="ms")
        nc.vector.tensor_copy(matrix_sb[:].bitcast(F32R), psM[:])
        ksum_sb = mspool.tile([M + 1, 1], F32, tag="ksum")
        nc.vector.tensor_scalar_mul(ksum_sb[M:M + 1, :].bitcast(F32R),
                                    onesP_sb[0:1, 0:1], float(NL))
        nc.vector.tensor_scalar_add(ksum_sb[0:M, :].bitcast(F32R), psKS[:],
                                    EPS)

        # ------- phases 4+5 interleaved -------
        for jc in range(32):
            psP = pp1.tile([128, 512], F32, tag="a", name="psP")
            nc.tensor.matmul(psP[:], _r(qn_t[:, ts(jc, 128)]),
                             _r(matrix_sb[:]), start=True, stop=True)
            psT = pp3.tile([128, 1], F32, tag="c", name="psT")
            nc.tensor.matmul(psT[:], qn_t[:, ts(jc, 128)],
                             ksum_sb[:], start=True, stop=True)
            sT = stpool.tile([128, 1], F32, tag="sT", name="sT")
            nc.vector.reciprocal(sT[:], psT[:])
            # ll'^T chunk via PE transposes
            psL = pp2.tile([128, 512], F32, tag="b", name="psL")
            for cb in range(4):
                nc.tensor.matmul(psL[:, ts(cb, 128)],
                                 ll_t[cb][:, ts(jc, 128)], eye_sb[:],
                                 is_transpose=True,
                                 start=True, stop=True,
                                 skip_group_check=True)
            # combined rhs tiles: rows 0:64 pscal half, rows 64:128 ll^T half
            comb_a = cbpool.tile([128, 512], F32, tag="comb_a", name="comb_a")
            nc.scalar.mul(comb_a[0:M, :].bitcast(F32R), psP[0:M, :],
                          sT[0:M, 0:1])
            nc.vector.tensor_copy(comb_a[M:128, :].bitcast(F32R),
                                  psL[0:M, :])
            comb_b = cbpool.tile([128, 512], F32, tag="comb_b", name="comb_b")
            nc.vector.tensor_scalar_mul(comb_b[0:M, :].bitcast(F32R),
                                        psP[M:128, :], sT[M:128, 0:1])
            nc.scalar.copy(comb_b[M:128, :].bitcast(F32R), psL[M:128, :])

            for wi in range(4):
                w = 4 * jc + wi
                comb = comb_a if wi < 2 else comb_b
                xw = xwpool.tile([128, 512], F32, tag="xw", name="xw")
                nc.sync.dma_start(
                    xw[:].rearrange("p (cb h) -> p cb h", cb=4),
                    d["xb"].rearrange("(cb p) n -> p cb n", p=128)[
                        :, :, w * 128:(w + 1) * 128])
                psO = pp1.tile([128, 512], F32, tag="a", name="psO")
                nc.tensor.matmul(psO[:], _r(scomb_r[:]), _r(comb[:]),
                                 start=True, stop=False,
                                 skip_group_check=True)
                for cb in range(4):
                    nc.tensor.matmul(psO[:, ts(cb, 128)],
                                     xw[:, ts(cb, 128)], eye_sb[:],
                                     is_transpose=True,
                                     start=False, stop=(cb == 3),
                                     skip_group_check=True)
                out_s = opool.tile([128, 512], F32, tag="outs", name="outs")
                if w % 2 == 0:
                    nc.vector.tensor_copy(out_s[:], psO[:])
                else:
                    nc.scalar.copy(out_s[:], psO[:])
                nc.sync.dma_start(d["out"][w * 128:(w + 1) * 128, :],
                                  out_s[:])


# ------------------------------------------------------------------
# host-side wrapper
# ------------------------------------------------------------------
_NC_CACHE = None


def _get_program():
    global _NC_CACHE
    if _NC_CACHE is None:
        _NC_CACHE = build_program()
    return _NC_CACHE


def _make_in_map(xb, wq, bq, wk, bk, wv, bv, gamma):
    dup = np.zeros((M, 128), dtype=np.float32)
    for j in range(M):
        dup[j, 2 * j] = 1.0
        dup[j, 2 * j + 1] = 1.0
    g = float(np.asarray(gamma).reshape(-1)[0])
    wqT = np.zeros((C, 128), dtype=np.float32)
    wqT[:, 0:M] = (0.5 * np.asarray(wq)).T
    scomb = np.concatenate([0.5 * g * dup, -0.25 * dup], axis=0)
    return {
        "xb": np.ascontiguousarray(np.asarray(xb).reshape(C, N)),
        "wqT": wqT,
        "wkT": np.ascontiguousarray((0.5 * np.asarray(wk)).T),
        "wvT": np.ascontiguousarray((0.5 * np.asarray(wv)).T),
        "bq": np.ascontiguousarray(np.asarray(bq).reshape(M, 1)),
        "bkb": np.ascontiguousarray(
            np.broadcast_to(np.asarray(bk)[None, :], (128, M))),
        "bvb": np.ascontiguousarray(
            np.broadcast_to(np.asarray(bv)[None, :], (128, C))),
        "eye": np.eye(128, dtype=np.float32),
        "scomb": np.ascontiguousarray(scomb),
        "onesP": np.ones((128, 128), dtype=np.float32),
    }


def kernel(x, y, gamma, gamma_y, wq, bq, wk, bk, wv, bv,
           wqy, bqy, wky, bky, wvy, bvy):
    x = np.asarray(x, dtype=np.float32)
    y = np.asarray(y, dtype=np.float32)
    B = x.shape[0]
    assert x.shape == (B, N, C), x.shape

    nc = _get_program()
    in_maps = []
    for b in range(B):
        in_maps.append(_make_in_map(x[b], wq, bq, wk, bk, wv, bv, gamma))
    for b in range(B):
        in_maps.append(_make_in_map(y[b], wqy, bqy, wky, bky, wvy, bvy,
                                    gamma_y))
    res = bass_utils.run_bass_kernel_spmd(
        nc, in_maps, core_ids=list(range(8)))
    out_x = np.stack([res.results[b]["out"] for b in range(B)])
    out_y = np.stack([res.results[B + b]["out"] for b in range(B)])
    return (out_x, out_y)
